# revision 1
# baseline (speedup 1.0000x reference)
"""Trainium2 Bass kernel for nn_CPCircuitLayer (sparse_attention).

Math identity used:
    out[b, n] = sum_r cp_w[r] * head_mode[h_n, r] * e1[i_n, r] * e2[j_n, r]
              = T[h_n, i_n, j_n]
where
    e1 = hidden @ W1.T, e2 = hidden @ W2.T          ([S, R])
    T[h] = (e1 * (head_mode[h] * cp_w)) @ e2.T       ([S, S] per head)

Since N = NH*S*S exactly enumerates the dense table, we compute the dense
T on-device with matmuls (no per-row gathers) and apply the (usually
identity) index gather on the host.

Sharding: hidT / w12T (projection operands) are replicated to all 8 cores;
the 16 heads are sharded 2-per-core. Host pre-transposes hidden -> [H, S]
and stacks W1/W2 -> [H, 2R] so the device kernel needs no on-chip
transposes: one 16-step accumulated matmul produces [e1^T; e2^T] stacked on
partitions, then per head a scale + [64,128]x[64,256] matmul emits T.
"""

import numpy as np

B, S, H, R, NH = 1, 256, 2048, 64, 16
N_CORES = 8
HPC = NH // N_CORES   # heads per core
KC = H // 128         # contraction chunks (16)
GRP = 4               # chunks per DMA group
NG = KC // GRP        # number of DMA groups
N_WARMUP = 2          # dummy matmuls to flip the HAM clock gate early

USE_F32R = False      # float32r matmuls: 1 cyc/row vs 4 for float32, ~2e-4 err

_PROG = None
LAST_RUN = None  # BassKernelResults of the most recent run (for profiling)


def _build_program():
    global _PROG
    if _PROG is not None:
        return _PROG

    import concourse.bacc as bacc
    import concourse.tile as tile
    from concourse import mybir
    from concourse.vector_clock import ScopedClock

    f32 = mybir.dt.float32
    mmdt = mybir.dt.float32r if USE_F32R else f32

    class SlimTileContext(tile.TileContext):
        """TileContext with a cheaper kernel-tail: drain + one all-engine
        barrier. The stock exit adds semaphore clears and a second barrier
        (~3-4us) that only matter if another kernel runs in the same NEFF."""

        def _drain_and_barrier(self, tick_clock, wait_clock):
            drain_inst = self.nc.sync.drain()
            wait_clock.add_sem_waits(
                drain_inst.ins, ScopedClock({None: tick_clock.global_clock})
            )
            self.nc.all_engine_barrier(sem_only=True)
            popped = self.nc._tile_sem_poison_stack.pop()
            assert popped is self._sem_poison

    nc = bacc.Bacc("TRN2", target_bir_lowering=False, debug=False,
                   num_devices=1)
    hidT = nc.declare_dram_parameter("hidT", [H, S], mmdt, isOutput=False)
    w12T = nc.declare_dram_parameter("w12T", [H, 2 * R], mmdt, isOutput=False)
    hmwT = nc.declare_dram_parameter("hmwT", [R, HPC], f32, isOutput=False)
    out = nc.declare_dram_parameter("out", [HPC * S, S], f32, isOutput=True)

    # Interleaved grouped views: within group g, partition p holds DRAM rows
    # g*512 + 4p + k (k = 0..3), so each partition's DMA read is one
    # contiguous 4KB (hid) / 2KB (w12) run. The matmul contraction only
    # needs lhsT and rhs to agree on the h <-> (p, k) mapping, which they do.
    hidT_v = hidT.rearrange("(g p k) s -> g p (k s)", p=128, k=GRP)
    w12T_v = w12T.rearrange("(g p k) m -> g p (k m)", p=128, k=GRP)

    with SlimTileContext(nc) as tc:
        with (
            tc.tile_pool(name="consts", bufs=1) as consts,
            tc.tile_pool(name="work", bufs=1) as work,
            tc.tile_pool(name="outp", bufs=4) as outp,
            tc.tile_pool(name="psum_e", bufs=1, space="PSUM") as psum_e,
            tc.tile_pool(name="psum_t", bufs=4, space="PSUM") as psum_t,
            tc.tile_pool(name="psum_w", bufs=1, space="PSUM") as psum_w,
        ):
            # PE warmup: the HAM clock gate keeps the PE at 1.2 GHz until it
            # has seen ~3.4us of sustained activity. Run dummy matmuls on a
            # zeroed scratch tile while the input DMAs stream so the real
            # chain runs at 2.4 GHz from its first instruction.
            wz = consts.tile([128, 512], mmdt, tag="warm_z")
            nc.gpsimd.memset(wz, 0.0)
            wps = psum_w.tile([128, 512], f32, tag="warm_ps")
            for _ in range(N_WARMUP):
                nc.tensor.matmul(wps, lhsT=wz[:, 0:128], rhs=wz,
                                 start=True, stop=True)

            # Alternate the two HWDGE issue queues (sync / scalar) between
            # the w and hid transfers of successive groups so both queues
            # carry ~half the bytes and group g's pair completes early.
            hid_tiles = []
            w_tiles = []
            for g in range(NG):
                e_w = nc.scalar if g % 2 == 0 else nc.sync
                e_h = nc.sync if g % 2 == 0 else nc.scalar
                wt = consts.tile([128, GRP, 2 * R], mmdt, tag=f"w{g}")
                e_w.dma_start(out=wt.rearrange("p k m -> p (k m)"),
                              in_=w12T_v[g])
                ht = consts.tile([128, GRP, S], mmdt, tag=f"hid{g}")
                e_h.dma_start(out=ht.rearrange("p k s -> p (k s)"),
                              in_=hidT_v[g])
                hid_tiles.append(ht)
                w_tiles.append(wt)

            hmw_sb = consts.tile([R, HPC], f32, tag="hmw")
            nc.scalar.dma_start(out=hmw_sb, in_=hmwT[:, :])

            # e12^T = [e1^T; e2^T] : [2R=128 partitions, S]
            e12_ps = psum_e.tile([128, S], f32, tag="e12")
            for g in range(NG):
                for k in range(GRP):
                    i = g * GRP + k
                    nc.tensor.matmul(e12_ps, lhsT=w_tiles[g][:, k, :],
                                     rhs=hid_tiles[g][:, k, :],
                                     start=(i == 0), stop=(i == KC - 1))

            e2t = work.tile([R, S], mmdt, tag="e2t")
            nc.vector.tensor_copy(out=e2t, in_=e12_ps[R:2 * R, :])

            # Per head: both i-chunk matmuls land in one [128, 2S] PSUM tile,
            # then a single wide copy and a single 256KB output DMA.
            out_v = out.rearrange("(h c p) s -> h p c s", p=128, c=S // 128)
            for h in range(HPC):
                # Split the scale per i-chunk so T-matmul ic launches as soon
                # as ITS half of s1 is written, not the full 256 columns.
                t_ps = psum_t.tile([128, 2 * S], f32, tag="t_ps")
                for ic in range(S // 128):
                    s1 = work.tile([R, 128], mmdt, tag=f"s1_{h}_{ic}")
                    nc.vector.tensor_scalar_mul(
                        out=s1, in0=e12_ps[0:R, ic * 128:(ic + 1) * 128],
                        scalar1=hmw_sb[:, h:h + 1])
                    nc.tensor.matmul(t_ps[:, ic * S:(ic + 1) * S],
                                     lhsT=s1, rhs=e2t, start=True, stop=True)
                o_sb = outp.tile([128, 2 * S], f32, tag="o_sb")
                nc.vector.tensor_copy(out=o_sb, in_=t_ps)
                nc.sync.dma_start(out=out_v[h, :, 0], in_=o_sb[:, 0:S])
                nc.scalar.dma_start(out=out_v[h, :, 1], in_=o_sb[:, S:2 * S])

    nc.compile()
    _PROG = nc
    return nc


def kernel(hidden_states, all_indices, W1, W2, head_mode, cp_w):
    global LAST_RUN
    from concourse.bass_utils import run_bass_kernel_spmd

    hidden = np.ascontiguousarray(np.asarray(hidden_states), dtype=np.float32)
    W1 = np.asarray(W1, dtype=np.float32)
    W2 = np.asarray(W2, dtype=np.float32)
    head_mode = np.asarray(head_mode, dtype=np.float32)
    cp_w = np.asarray(cp_w, dtype=np.float32)
    ai = np.asarray(all_indices)

    assert hidden.shape == (B, S, H), hidden.shape
    assert ai.shape[1] == 3

    nc = _build_program()

    hidT = np.ascontiguousarray(hidden[0].T)                       # [H, S]
    w12T = np.ascontiguousarray(np.concatenate([W1, W2], 0).T)     # [H, 2R]
    hmw = head_mode * cp_w                                         # [NH, R]

    in_maps = [
        {
            "hidT": hidT,
            "w12T": w12T,
            "hmwT": np.ascontiguousarray(hmw[c * HPC:(c + 1) * HPC].T),
        }
        for c in range(N_CORES)
    ]
    res = run_bass_kernel_spmd(nc, in_maps, core_ids=list(range(N_CORES)))
    LAST_RUN = res

    T = np.concatenate(
        [np.asarray(res.results[c]["out"]).reshape(HPC, S, S)
         for c in range(N_CORES)], axis=0)                         # [NH, S, S]

    n = ai.shape[0]
    flat = (ai[:, 0].astype(np.int64) * S + ai[:, 1].astype(np.int64)) * S \
        + ai[:, 2].astype(np.int64)
    if n == NH * S * S and np.array_equal(flat, np.arange(n, dtype=np.int64)):
        out = T.reshape(B, NH, S, S)
    else:
        out = np.take(T.reshape(-1), flat).reshape(B, NH, S, S)
    return np.ascontiguousarray(out, dtype=np.float32)



# revision 2
# speedup vs baseline: 1.3956x; 1.3956x over previous
"""Trainium2 Bass kernel for nn_CPCircuitLayer (sparse_attention).

Math identity:
    out[b, n] = sum_r cp_w[r] * head_mode[h_n, r] * e1[i_n, r] * e2[j_n, r]
              = T[h_n, i_n, j_n]
where
    e1 = hidden @ W1.T, e2 = hidden @ W2.T          ([S, R])
    T[h] = (e1 * (head_mode[h] * cp_w)) @ e2.T       ([S, S] per head)

N = NH*S*S enumerates the dense table, so we compute dense T on-device and
apply the (usually identity) index gather on the host.

Sharding: inputs replicated to all 8 cores; the 16 heads sharded 2-per-core.

Device-side layout (all bf16 to halve DMA bytes and run 1-cycle/row
matmuls; PSUM accumulation stays f32, end-to-end max rel err ~8e-3 vs the
2e-2 gate):
  packed[h, :] = [W1T[h, 0:64] | W2T[h, 64:128] | hidT[h, 0:256]]  [2048, 384]
  Four DMA groups of 4 contraction chunks stream on the sync/scalar HWDGE
  queues; each group's chunk k carries BOTH matmul operands, so chunk
  availability advances uniformly.  16-step accumulated matmul produces
  [e1^T; e2^T] stacked on partitions; per head a scale + [64,128]x[64,256]
  matmul emits T, copied to SBUF as bf16 and DMA'd out (host converts back
  to f32).
"""

import numpy as np

B, S, H, R, NH = 1, 256, 2048, 64, 16
N_CORES = 8
HPC = NH // N_CORES   # heads per core
KC = H // 128         # contraction chunks (16)
GRP = 4               # chunks per DMA group
NG = KC // GRP        # number of DMA groups
PK = 2 * R + S        # packed row length (384)
N_WARMUP = 4          # bf16 warmup matmuls to flip the HAM clock gate

_PROG = None
LAST_RUN = None  # BassKernelResults of the most recent run (for profiling)


def _build_program():
    global _PROG
    if _PROG is not None:
        return _PROG

    import concourse.bacc as bacc
    import concourse.tile as tile
    from concourse import mybir
    from concourse.vector_clock import ScopedClock

    f32 = mybir.dt.float32
    bf16 = mybir.dt.bfloat16

    class SlimTileContext(tile.TileContext):
        """TileContext with a cheaper kernel-tail: drain + one all-engine
        barrier. The stock exit adds semaphore clears and a second barrier
        that only matter if another kernel runs in the same NEFF."""

        def _drain_and_barrier(self, tick_clock, wait_clock):
            drain_inst = self.nc.sync.drain()
            wait_clock.add_sem_waits(
                drain_inst.ins, ScopedClock({None: tick_clock.global_clock})
            )
            self.nc.all_engine_barrier(sem_only=True)
            popped = self.nc._tile_sem_poison_stack.pop()
            assert popped is self._sem_poison

    nc = bacc.Bacc("TRN2", target_bir_lowering=False, debug=False,
                   num_devices=1)
    packed = nc.declare_dram_parameter("packed", [H, PK], bf16, isOutput=False)
    hmwT = nc.declare_dram_parameter("hmwT", [R, HPC], f32, isOutput=False)
    out = nc.declare_dram_parameter("out", [HPC * 128, 2 * S], bf16,
                                    isOutput=True)

    # Interleaved grouped view: within group g, partition p holds DRAM rows
    # g*512 + 4p + k (k = 0..3), so each partition's DMA read is one
    # contiguous 3KB run. The matmul contraction only needs lhsT and rhs to
    # agree on the h <-> (p, k) mapping, which they do.
    packed_v = packed.rearrange("(g p k) m -> g p (k m)", p=128, k=GRP)
    out_v = out.rearrange("(h p) m -> h p m", p=128)

    with SlimTileContext(nc) as tc:
        with (
            tc.tile_pool(name="consts", bufs=1) as consts,
            tc.tile_pool(name="work", bufs=1) as work,
            tc.tile_pool(name="outp", bufs=2) as outp,
            tc.tile_pool(name="psum_e", bufs=1, space="PSUM") as psum_e,
            tc.tile_pool(name="psum_t", bufs=2, space="PSUM") as psum_t,
            tc.tile_pool(name="psum_w", bufs=1, space="PSUM") as psum_w,
        ):
            # PE warmup: HAM keeps the PE at 1.2 GHz until ~3.4us of
            # sustained activity; dummy matmuls run while input DMAs stream.
            wz = consts.tile([128, 512], bf16, tag="warm_z")
            nc.gpsimd.memset(wz, 0.0)
            wps = psum_w.tile([128, 512], f32, tag="warm_ps")
            for _ in range(N_WARMUP):
                nc.tensor.matmul(wps, lhsT=wz[:, 0:128], rhs=wz,
                                 start=True, stop=True)

            pk_tiles = []
            for g in range(NG):
                eng = nc.sync if g % 2 == 0 else nc.scalar
                pt = consts.tile([128, GRP, PK], bf16, tag=f"pk{g}")
                eng.dma_start(out=pt.rearrange("p k m -> p (k m)"),
                              in_=packed_v[g])
                pk_tiles.append(pt)

            hmw_sb = consts.tile([R, HPC], f32, tag="hmw")
            nc.gpsimd.dma_start(out=hmw_sb, in_=hmwT[:, :])

            # e12^T = [e1^T; e2^T] : [2R=128 partitions, S]
            e12_ps = psum_e.tile([128, S], f32, tag="e12")
            for g in range(NG):
                for k in range(GRP):
                    i = g * GRP + k
                    nc.tensor.matmul(e12_ps,
                                     lhsT=pk_tiles[g][:, k, 0:2 * R],
                                     rhs=pk_tiles[g][:, k, 2 * R:PK],
                                     start=(i == 0), stop=(i == KC - 1))

            e2t = work.tile([R, S], bf16, tag="e2t")
            nc.vector.tensor_copy(out=e2t, in_=e12_ps[R:2 * R, :])

            # Per head: both i-chunk matmuls land in one [128, 2S] PSUM
            # tile, then one wide bf16 copy and one 128KB output DMA.
            for h in range(HPC):
                t_ps = psum_t.tile([128, 2 * S], f32, tag="t_ps")
                for ic in range(2):
                    s1 = work.tile([R, 128], bf16, tag=f"s1_{h}_{ic}")
                    nc.vector.tensor_scalar_mul(
                        out=s1, in0=e12_ps[0:R, ic * 128:(ic + 1) * 128],
                        scalar1=hmw_sb[:, h:h + 1])
                    nc.tensor.matmul(t_ps[:, ic * S:(ic + 1) * S],
                                     lhsT=s1, rhs=e2t, start=True, stop=True)
                o_sb = outp.tile([128, 2 * S], bf16, tag="o_sb")
                nc.vector.tensor_copy(out=o_sb, in_=t_ps)
                eng = nc.sync if h % 2 == 0 else nc.scalar
                eng.dma_start(out=out_v[h], in_=o_sb)

    nc.compile()
    _PROG = nc
    return nc


def kernel(hidden_states, all_indices, W1, W2, head_mode, cp_w):
    global LAST_RUN
    import ml_dtypes
    from concourse.bass_utils import run_bass_kernel_spmd

    bf = ml_dtypes.bfloat16
    hidden = np.asarray(hidden_states, dtype=np.float32)
    W1 = np.asarray(W1, dtype=np.float32)
    W2 = np.asarray(W2, dtype=np.float32)
    head_mode = np.asarray(head_mode, dtype=np.float32)
    cp_w = np.asarray(cp_w, dtype=np.float32)
    ai = np.asarray(all_indices)

    assert hidden.shape == (B, S, H), hidden.shape
    assert ai.shape[1] == 3

    nc = _build_program()

    # packed rows: [W1T | W2T | hidT] -> [H, 384] bf16
    packed = np.empty((H, PK), dtype=bf)
    packed[:, 0:R] = W1.T.astype(bf)
    packed[:, R:2 * R] = W2.T.astype(bf)
    packed[:, 2 * R:PK] = hidden[0].T.astype(bf)
    packed = np.ascontiguousarray(packed)
    hmw = head_mode * cp_w                                         # [NH, R]

    in_maps = [
        {
            "packed": packed,
            "hmwT": np.ascontiguousarray(hmw[c * HPC:(c + 1) * HPC].T),
        }
        for c in range(N_CORES)
    ]
    res = run_bass_kernel_spmd(nc, in_maps, core_ids=list(range(N_CORES)))
    LAST_RUN = res

    # out[h, p, ic*S + j] = T[h, ic*128 + p, j]
    T = np.concatenate(
        [np.asarray(res.results[c]["out"]).astype(np.float32)
         .reshape(HPC, 128, 2, S).transpose(0, 2, 1, 3).reshape(HPC, S, S)
         for c in range(N_CORES)], axis=0)                         # [NH, S, S]

    n = ai.shape[0]
    flat = (ai[:, 0].astype(np.int64) * S + ai[:, 1].astype(np.int64)) * S \
        + ai[:, 2].astype(np.int64)
    if n == NH * S * S and np.array_equal(flat, np.arange(n, dtype=np.int64)):
        out = T.reshape(B, NH, S, S)
    else:
        out = np.take(T.reshape(-1), flat).reshape(B, NH, S, S)
    return np.ascontiguousarray(out, dtype=np.float32)


# revision 4
# speedup vs baseline: 1.5097x; 1.0817x over previous
"""Trainium2 Bass kernel for nn_CPCircuitLayer (sparse_attention).

Math identity:
    out[b, n] = sum_r cp_w[r] * head_mode[h_n, r] * e1[i_n, r] * e2[j_n, r]
              = T[h_n, i_n, j_n]
where
    e1 = hidden @ W1.T, e2 = hidden @ W2.T          ([S, R])
    T[h] = (e1 * (head_mode[h] * cp_w)) @ e2.T       ([S, S] per head)

N = NH*S*S enumerates the dense table, so we compute dense T on-device and
apply the (usually identity) index gather on the host.

Sharding: inputs replicated to all 8 cores; the 16 heads sharded 2-per-core.

Device-side layout (all bf16 to halve DMA bytes and run 1-cycle/row
matmuls; PSUM accumulation stays f32, end-to-end max rel err ~8e-3 vs the
2e-2 gate):
  packed[h, :] = [W1T[h, 0:64] | W2T[h, 64:128] | hidT[h, 0:256]]  [2048, 384]
  Four DMA groups of 4 contraction chunks stream on the sync/scalar HWDGE
  queues; each group's chunk k carries BOTH matmul operands, so chunk
  availability advances uniformly.  16-step accumulated matmul produces
  [e1^T; e2^T] stacked on partitions; per head a scale + [64,128]x[64,256]
  matmul emits T, copied to SBUF as bf16 and DMA'd out (host converts back
  to f32).
"""

import numpy as np

B, S, H, R, NH = 1, 256, 2048, 64, 16
N_CORES = 8
HPC = NH // N_CORES   # heads per core
KC = H // 128         # contraction chunks (16)
GRP = 4               # chunks per DMA group
NG = KC // GRP        # number of DMA groups
PK = 2 * R + S        # packed row length (384)
N_WARMUP = 8          # bf16 warmup matmuls to flip the HAM clock gate

_PROG = None
LAST_RUN = None  # BassKernelResults of the most recent run (for profiling)


def _build_program():
    global _PROG
    if _PROG is not None:
        return _PROG

    import concourse.bacc as bacc
    import concourse.tile as tile
    from concourse import mybir
    from concourse.vector_clock import ScopedClock

    f32 = mybir.dt.float32
    bf16 = mybir.dt.bfloat16

    class SlimTileContext(tile.TileContext):
        """TileContext with a cheaper kernel-tail: drain + one all-engine
        barrier. The stock exit adds semaphore clears and a second barrier
        that only matter if another kernel runs in the same NEFF."""

        def _drain_and_barrier(self, tick_clock, wait_clock):
            drain_inst = self.nc.sync.drain()
            wait_clock.add_sem_waits(
                drain_inst.ins, ScopedClock({None: tick_clock.global_clock})
            )
            self.nc.all_engine_barrier(sem_only=True)
            popped = self.nc._tile_sem_poison_stack.pop()
            assert popped is self._sem_poison

    nc = bacc.Bacc("TRN2", target_bir_lowering=False, debug=False,
                   num_devices=1)
    packed = nc.declare_dram_parameter("packed", [H, PK], bf16, isOutput=False)
    hmwT = nc.declare_dram_parameter("hmwT", [R, HPC], f32, isOutput=False)
    out = nc.declare_dram_parameter("out", [HPC * 128, 2 * S], bf16,
                                    isOutput=True)

    # Interleaved grouped view: within group g, partition p holds DRAM rows
    # g*512 + 4p + k (k = 0..3), so each partition's DMA read is one
    # contiguous 3KB run. The matmul contraction only needs lhsT and rhs to
    # agree on the h <-> (p, k) mapping, which they do.
    packed_v = packed.rearrange("(g p k) m -> g p (k m)", p=128, k=GRP)
    out_v = out.rearrange("(h p) m -> h p m", p=128)

    with SlimTileContext(nc) as tc:
        with (
            tc.tile_pool(name="consts", bufs=1) as consts,
            tc.tile_pool(name="work", bufs=1) as work,
            tc.tile_pool(name="outp", bufs=2) as outp,
            tc.tile_pool(name="psum_e", bufs=1, space="PSUM") as psum_e,
            tc.tile_pool(name="psum_t", bufs=2, space="PSUM") as psum_t,
            tc.tile_pool(name="psum_w", bufs=1, space="PSUM") as psum_w,
        ):
            # PE warmup: HAM keeps the PE at 1.2 GHz until ~3.4us of
            # sustained activity; dummy matmuls run while input DMAs stream.
            wz = consts.tile([128, 512], bf16, tag="warm_z")
            nc.gpsimd.memset(wz, 0.0)
            wps = psum_w.tile([128, 512], f32, tag="warm_ps")
            for _ in range(N_WARMUP):
                nc.tensor.matmul(wps, lhsT=wz[:, 0:128], rhs=wz,
                                 start=True, stop=True)

            pk_tiles = []
            for g in range(NG):
                eng = nc.sync if g % 2 == 0 else nc.scalar
                pt = consts.tile([128, GRP, PK], bf16, tag=f"pk{g}")
                eng.dma_start(out=pt.rearrange("p k m -> p (k m)"),
                              in_=packed_v[g])
                pk_tiles.append(pt)

            # hmw is tiny and only needed post-chain; issue it after the
            # input groups so it doesn't delay their descriptor generation
            # on the shared HWDGE.
            hmw_sb = consts.tile([R, HPC], f32, tag="hmw")
            nc.gpsimd.dma_start(out=hmw_sb, in_=hmwT[:, :])

            # e12^T = [e1^T; e2^T] : [2R=128 partitions, S]
            e12_ps = psum_e.tile([128, S], f32, tag="e12")
            for g in range(NG):
                for k in range(GRP):
                    i = g * GRP + k
                    nc.tensor.matmul(e12_ps,
                                     lhsT=pk_tiles[g][:, k, 0:2 * R],
                                     rhs=pk_tiles[g][:, k, 2 * R:PK],
                                     start=(i == 0), stop=(i == KC - 1))

            # Post-chain tail, split across the Act (nc.scalar) and DVE
            # (nc.vector) engines so casts and scales run concurrently:
            #   Act: e2t copy, then head-0's two output half-casts
            #   DVE: the four s1 scales, then head-1's two half-casts
            Copy = mybir.ActivationFunctionType.Copy
            e2t = work.tile([R, S], bf16, tag="e2t")
            nc.scalar.activation(e2t, e12_ps[R:2 * R, :], Copy)

            t_pss = []
            for h in range(HPC):
                t_ps = psum_t.tile([128, 2 * S], f32, tag=f"t_ps{h}")
                for ic in range(2):
                    s1 = work.tile([R, 128], bf16, tag=f"s1_{h}_{ic}")
                    nc.vector.tensor_scalar_mul(
                        out=s1, in0=e12_ps[0:R, ic * 128:(ic + 1) * 128],
                        scalar1=hmw_sb[:, h:h + 1])
                    nc.tensor.matmul(t_ps[:, ic * S:(ic + 1) * S],
                                     lhsT=s1, rhs=e2t, start=True, stop=True)
                t_pss.append(t_ps)

            # head 0: Act casts both halves as each matmul lands; one DMA
            # issued from the scalar queue right after.
            o0 = outp.tile([128, 2 * S], bf16, tag="o0")
            nc.scalar.activation(o0[:, 0:S], t_pss[0][:, 0:S], Copy)
            nc.scalar.activation(o0[:, S:2 * S], t_pss[0][:, S:2 * S], Copy)
            nc.scalar.dma_start(out=out_v[0], in_=o0)

            # head 1: DVE casts both halves; first half DMA'd from sync as
            # soon as it lands, second from gpsimd (engine-cheap SWDGE).
            o1 = outp.tile([128, 2 * S], bf16, tag="o1")
            nc.vector.tensor_copy(out=o1[:, 0:S], in_=t_pss[1][:, 0:S])
            nc.sync.dma_start(out=out_v[1][:, 0:S], in_=o1[:, 0:S])
            nc.vector.tensor_copy(out=o1[:, S:2 * S], in_=t_pss[1][:, S:2 * S])
            nc.gpsimd.dma_start(out=out_v[1][:, S:2 * S], in_=o1[:, S:2 * S])

    nc.compile()
    _PROG = nc
    return nc


def kernel(hidden_states, all_indices, W1, W2, head_mode, cp_w):
    global LAST_RUN
    import ml_dtypes
    from concourse.bass_utils import run_bass_kernel_spmd

    bf = ml_dtypes.bfloat16
    hidden = np.asarray(hidden_states, dtype=np.float32)
    W1 = np.asarray(W1, dtype=np.float32)
    W2 = np.asarray(W2, dtype=np.float32)
    head_mode = np.asarray(head_mode, dtype=np.float32)
    cp_w = np.asarray(cp_w, dtype=np.float32)
    ai = np.asarray(all_indices)

    assert hidden.shape == (B, S, H), hidden.shape
    assert ai.shape[1] == 3

    nc = _build_program()

    # packed rows: [W1T | W2T | hidT] -> [H, 384] bf16
    packed = np.empty((H, PK), dtype=bf)
    packed[:, 0:R] = W1.T.astype(bf)
    packed[:, R:2 * R] = W2.T.astype(bf)
    packed[:, 2 * R:PK] = hidden[0].T.astype(bf)
    packed = np.ascontiguousarray(packed)
    hmw = head_mode * cp_w                                         # [NH, R]

    in_maps = [
        {
            "packed": packed,
            "hmwT": np.ascontiguousarray(hmw[c * HPC:(c + 1) * HPC].T),
        }
        for c in range(N_CORES)
    ]
    res = run_bass_kernel_spmd(nc, in_maps, core_ids=list(range(N_CORES)))
    LAST_RUN = res

    # out[h, p, ic*S + j] = T[h, ic*128 + p, j]
    T = np.concatenate(
        [np.asarray(res.results[c]["out"]).astype(np.float32)
         .reshape(HPC, 128, 2, S).transpose(0, 2, 1, 3).reshape(HPC, S, S)
         for c in range(N_CORES)], axis=0)                         # [NH, S, S]

    n = ai.shape[0]
    flat = (ai[:, 0].astype(np.int64) * S + ai[:, 1].astype(np.int64)) * S \
        + ai[:, 2].astype(np.int64)
    if n == NH * S * S and np.array_equal(flat, np.arange(n, dtype=np.int64)):
        out = T.reshape(B, NH, S, S)
    else:
        out = np.take(T.reshape(-1), flat).reshape(B, NH, S, S)
    return np.ascontiguousarray(out, dtype=np.float32)
